# revision 10
# baseline (speedup 1.0000x reference)
"""Multi-head attention (B=2, S=2048, D=1024, H=16) on 8 trn2 NeuronCores.

Sharding: core c handles batch (c // 4) and heads 4*(c % 4) .. +4 (tensor
parallel over heads, data parallel over batch). Each core computes its 4
heads' Q/K/V projections, the full attention-weight matrix for those heads
(written to HBM as output), and its partial output projection (row-sharded
W_o); the cross-core reduction ("all-reduce after W_o") happens on the host
at unshard time, where the b_v/b_o bias terms are also folded in (exact:
per-row softmax normalization commutes with the W_o contraction, and the
V-bias contributes (W_o @ b_v) to every output row).

Matmuls run in float32r (single-pass fp32, ~1.2e-4 rounding) which streams
at 1 cycle/row for free dims >= 256 vs 4 cycles/row for exact fp32.

Causal masking is done on the tensor engine: an identity x pattern-tile
matmul accumulates -1e30 into masked PSUM score entries before exp (exp
then underflows to exactly 0). Fully masked tiles are skipped and never
written: ExternalOutput buffers are pre-zeroed by the run contract.
"""

import math
from contextlib import ExitStack

import ml_dtypes
import numpy as np

import concourse.bass as bass
import concourse.mybir as mybir
import concourse.tile as tile
from concourse.bass_utils import run_bass_kernel_spmd

D_MODEL = 1024
NUM_HEADS = 16
D_K = 64
B = 2
S = 2048
N_CORES = 8
HPC = 4  # heads per core
C_PC = HPC * D_K  # 256 projected channels per core
SCALE = 1.0 / math.sqrt(D_K)

NB = S // 512  # 4 column blocks of 512
NT = S // 128  # 16 row tiles of 128
KT = D_MODEL // 128  # 8 contraction tiles

F32 = mybir.dt.float32
F32R = mybir.dt.float32r
BF16 = mybir.dt.bfloat16

TRACE = False  # set by test.py for profiling runs
_TRACE_RESULT = {}


def _install_ntff_hook():
    """This image's antenv lacks axon_hooks; synthesize it from the PJRT
    .so's profiling C ABI so run_bass_kernel_spmd's trace path works."""
    import contextlib
    import ctypes
    import os
    import sys
    import types

    try:
        from antenv.axon_hooks import get_axon_ntff_profile_hook  # noqa: F401

        return True
    except ImportError:
        pass
    so = "/opt/axon/libaxon_pjrt.so"
    if not os.path.exists(so):
        return False
    lib = ctypes.CDLL(so)
    if not hasattr(lib, "axon_start_nrt_profile"):
        return False
    lib.axon_start_nrt_profile.argtypes = [
        ctypes.POINTER(ctypes.c_int64),
        ctypes.c_size_t,
    ]
    lib.axon_start_nrt_profile.restype = ctypes.c_int64
    lib.axon_stop_nrt_profile.argtypes = [ctypes.c_char_p]
    lib.axon_stop_nrt_profile.restype = ctypes.c_int64

    @contextlib.contextmanager
    def _hook(output_dir, device_ids):
        import jax

        jax.devices()
        if device_ids:
            ids = (ctypes.c_int64 * len(device_ids))(*device_ids)
            rc = lib.axon_start_nrt_profile(ids, len(device_ids))
        else:
            rc = lib.axon_start_nrt_profile(None, 0)
        if rc != 0:
            raise RuntimeError(f"axon_start_nrt_profile rc={rc}")
        try:
            yield
        finally:
            n = lib.axon_stop_nrt_profile(str(output_dir).encode())
            print(f"ntff profile: {n} file(s) -> {output_dir}", flush=True)

    mod = types.ModuleType("antenv.axon_hooks")
    mod.get_axon_ntff_profile_hook = lambda: _hook
    mod.set_axon_ntff_profile_hook = lambda h: None
    import antenv

    antenv.axon_hooks = mod
    sys.modules["antenv.axon_hooks"] = mod

    # zero-egress container: don't try to upload trace artifacts
    import concourse.bass_utils as bu

    bu.upload_artifacts = lambda tmpdir: f"local://{tmpdir}"
    return True

MM_DT = F32R  # flip to F32 if f32r precision ever proves insufficient


def _split_excess_waits(nc, max_waits=1):
    """walrus in this toolchain rejects instructions with more than one
    sync-wait (f32/f32r matmuls fail at 2; the Tile tail drain at 6).
    Move excess waits onto no-fuse NOPs just before the instruction on the
    same engine stream; per-engine order is preserved so this is exact."""
    for f in nc.m.functions:
        for blk in f.blocks:
            insts = blk.instructions
            out = []
            dirty = False
            for inst in insts:
                si = inst.sync_info
                if si is not None and len(si.on_wait) > max_waits:
                    waits = list(si.on_wait)
                    excess, kept = waits[:-max_waits], waits[-max_waits:]
                    for k in range(0, len(excess), max_waits):
                        nop = mybir.InstNoOp(name=f"I-{nc.next_id()}", ins=[], outs=[])
                        nop.engine = inst.engine
                        nop.bass_nofuse = True
                        nop.text_hint = "waitsplit"
                        nop.sync_info = mybir.SyncInfo(
                            on_wait=excess[k : k + max_waits], on_update=[]
                        )
                        nc.register_instruction(nop, overwrite=True)
                        out.append(nop)
                    inst.sync_info = mybir.SyncInfo(
                        on_wait=kept, on_update=list(si.on_update)
                    )
                    dirty = True
                out.append(inst)
            if dirty:
                blk.instructions = out


def _valid_jtiles(ib, mode):
    """128-wide key tiles that intersect the unmasked region for query block
    ib (512 queries)."""
    if mode == "causal":
        return list(range(4 * ib + 4))
    return list(range(NT))


def _build(mode):
    """mode: 'causal' | 'dense' | 'general'."""
    nc = bass.Bass("TRN2", target_bir_lowering=False, debug=False)

    qT = nc.dram_tensor("qT", [D_MODEL, S], MM_DT, kind="ExternalInput").ap()
    kT = nc.dram_tensor("kT", [D_MODEL, S], MM_DT, kind="ExternalInput").ap()
    vT = nc.dram_tensor("vT", [D_MODEL, S], MM_DT, kind="ExternalInput").ap()
    wqT = nc.dram_tensor("wqT", [D_MODEL, C_PC], MM_DT, kind="ExternalInput").ap()
    wkT = nc.dram_tensor("wkT", [D_MODEL, C_PC], MM_DT, kind="ExternalInput").ap()
    wvT = nc.dram_tensor("wvT", [D_MODEL, C_PC], MM_DT, kind="ExternalInput").ap()
    woT = nc.dram_tensor("woT", [C_PC, D_MODEL], MM_DT, kind="ExternalInput").ap()
    bq = nc.dram_tensor("bq", [C_PC, 1], F32, kind="ExternalInput").ap()
    bk = nc.dram_tensor("bk", [C_PC, 1], F32, kind="ExternalInput").ap()
    ident = nc.dram_tensor("ident", [128, 128], BF16, kind="ExternalInput").ap()
    if mode == "causal":
        patt1 = nc.dram_tensor("patt1", [4, 128, 512], BF16, kind="ExternalInput").ap()
        patt2 = nc.dram_tensor("patt2", [4, 128, 512], BF16, kind="ExternalInput").ap()
    elif mode == "general":
        mb1 = nc.dram_tensor("mb1", [S, S], BF16, kind="ExternalInput").ap()
        mb2 = nc.dram_tensor("mb2", [S, S], BF16, kind="ExternalInput").ap()

    attn_out = nc.dram_tensor("attn_out", [HPC, S, S], F32, kind="ExternalOutput").ap()
    out_p = nc.dram_tensor("out_p", [S, D_MODEL], F32, kind="ExternalOutput").ap()

    with tile.TileContext(nc) as tc, ExitStack() as ctx:
        const = ctx.enter_context(tc.tile_pool(name="const", bufs=1))
        resident = ctx.enter_context(tc.tile_pool(name="resident", bufs=1))

        # --- constants / weights ---
        wq_sb = const.tile([128, KT, C_PC], MM_DT)
        wk_sb = const.tile([128, KT, C_PC], MM_DT)
        wv_sb = const.tile([128, KT, C_PC], MM_DT)
        nc.sync.dma_start(wq_sb[:], wqT.rearrange("(kt p) c -> p kt c", p=128))
        nc.sync.dma_start(wk_sb[:], wkT.rearrange("(kt p) c -> p kt c", p=128))
        nc.sync.dma_start(wv_sb[:], wvT.rearrange("(kt p) c -> p kt c", p=128))
        wo_sb = const.tile([64, HPC, D_MODEL], MM_DT)
        nc.sync.dma_start(wo_sb[:], woT.rearrange("(h c) e -> c h e", c=64))
        bq_sb = const.tile([128, 2], F32)
        bk_sb = const.tile([128, 2], F32)
        nc.sync.dma_start(bq_sb[:], bq.rearrange("(m p) one -> p (m one)", p=128))
        nc.sync.dma_start(bk_sb[:], bk.rearrange("(m p) one -> p (m one)", p=128))
        ident_sb = const.tile([128, 128], BF16)
        nc.sync.dma_start(ident_sb[:], ident)
        if mode == "causal":
            patt1_sb = const.tile([128, 4, 512], BF16)
            patt2_sb = const.tile([128, 4, 512], BF16)
            nc.sync.dma_start(patt1_sb[:], patt1.rearrange("k p j -> p k j"))
            nc.sync.dma_start(patt2_sb[:], patt2.rearrange("k p j -> p k j"))

        # --- resident activations ---
        QT_sb = [resident.tile([128, S], MM_DT, name=f"QT{m}") for m in range(2)]
        KT_sb = [resident.tile([128, S], MM_DT, name=f"KT{m}") for m in range(2)]
        V_sb = resident.tile([128, NT, HPC, D_K], MM_DT)
        UT_sb = [resident.tile([64, S], MM_DT, name=f"UT{h}") for h in range(HPC)]
        # 1/rowsum per (query, i-tile, head), partition layout, from C4
        rinvT_sb = resident.tile([128, NT, HPC], F32)

        def head_q(h, cols):  # [64, w] slice of Q^T for head h
            return QT_sb[h // 2][(h % 2) * 64 : (h % 2) * 64 + 64, cols]

        def head_k(h, cols):
            return KT_sb[h // 2][(h % 2) * 64 : (h % 2) * 64 + 64, cols]

        # ================= phase B: projections =================
        with (
            tc.tile_pool(name="stage", bufs=2) as stage,
            tc.tile_pool(name="psB", bufs=1, space="PSUM") as psB,
        ):
            for nb in range(NB):
                cols = slice(nb * 512, nb * 512 + 512)
                for quarter in range(4):  # kt in four pairs
                    kts = range(quarter * 2, quarter * 2 + 2)
                    q_st = stage.tile([128, 2, 512], MM_DT, name="q_st")
                    k_st = stage.tile([128, 2, 512], MM_DT, name="k_st")
                    v_st = stage.tile([128, 2, 512], MM_DT, name="v_st")
                    src = qT.rearrange("(kt p) s -> p kt s", p=128)
                    nc.sync.dma_start(q_st[:], src[:, quarter * 2 : quarter * 2 + 2, cols])
                    src = kT.rearrange("(kt p) s -> p kt s", p=128)
                    nc.sync.dma_start(k_st[:], src[:, quarter * 2 : quarter * 2 + 2, cols])
                    src = vT.rearrange("(kt p) s -> p kt s", p=128)
                    nc.sync.dma_start(v_st[:], src[:, quarter * 2 : quarter * 2 + 2, cols])
                    if quarter == 0:
                        psQ = [
                            psB.tile([128, 512], F32, name="psQ", bufs=2)
                            for _ in range(2)
                        ]
                        psK = [
                            psB.tile([128, 512], F32, name="psK", bufs=2)
                            for _ in range(2)
                        ]
                        psV = [
                            psB.tile([128, 256], F32, name="psV", bufs=4)
                            for _ in range(4)
                        ]
                    for i, kt in enumerate(kts):
                        first = kt == 0
                        last = kt == KT - 1
                        for m in range(2):
                            mc = slice(m * 128, m * 128 + 128)
                            nc.tensor.matmul(
                                psQ[m][:], wq_sb[:, kt, mc], q_st[:, i, :],
                                start=first, stop=last,
                            )
                            nc.tensor.matmul(
                                psK[m][:], wk_sb[:, kt, mc], k_st[:, i, :],
                                start=first, stop=last,
                            )
                        for jl in range(4):
                            nc.tensor.matmul(
                                psV[jl][:],
                                v_st[:, i, jl * 128 : jl * 128 + 128],
                                wv_sb[:, kt, :],
                                start=first, stop=last,
                            )
                for m in range(2):
                    nc.scalar.add(QT_sb[m][:, cols], psQ[m][:], bq_sb[:, m : m + 1])
                    nc.scalar.add(KT_sb[m][:, cols], psK[m][:], bk_sb[:, m : m + 1])
                for jl in range(4):
                    jt = nb * 4 + jl
                    nc.vector.tensor_copy(
                        V_sb[:, jt, :, 0:D_K],
                        psV[jl][:].rearrange("p (h c) -> p h c", h=HPC),
                    )

        # ================= phases C & D =================
        with (
            tc.tile_pool(name="ps512", bufs=4, space="PSUM") as ps512,
            tc.tile_pool(name="psU", bufs=2, space="PSUM") as psUp,
            tc.tile_pool(name="psOP", bufs=2, space="PSUM") as psOPp,
            tc.tile_pool(name="expp", bufs=4) as expp,
            tc.tile_pool(name="ppool", bufs=2) as ppool,
            tc.tile_pool(name="rpool", bufs=8) as rpool,
            tc.tile_pool(name="mbst", bufs=4) as mbst,
            tc.tile_pool(name="obuf", bufs=2) as obuf,
        ):
            for ib in range(NB):
                icols = slice(ib * 512, ib * 512 + 512)
                for h in range(HPC):
                    # ---- C1/C2: S^T -> exp -> U^T (unnormalized) ----
                    jts = _valid_jtiles(ib, mode)
                    psU = psUp.tile([64, 512], F32, name="psU")
                    for n, u in enumerate(jts):
                        diag = mode == "causal" and u >= 4 * ib
                        psST = ps512.tile([128, 512], F32, name="psST")
                        nc.tensor.matmul(
                            psST[:],
                            head_k(h, slice(u * 128, u * 128 + 128)),
                            head_q(h, icols),
                            start=True,
                            stop=not (diag or mode == "general"),
                        )
                        if diag:
                            nc.tensor.matmul(
                                psST[:], ident_sb[:], patt2_sb[:, u - 4 * ib, :],
                                start=False, stop=True,
                            )
                        elif mode == "general":
                            mt = mbst.tile([128, 512], BF16, name="mt")
                            nc.sync.dma_start(
                                mt[:], mb2[u * 128 : u * 128 + 128, icols]
                            )
                            nc.tensor.matmul(
                                psST[:], ident_sb[:], mt[:], start=False, stop=True
                            )
                        expst = expp.tile([128, 512], MM_DT, name="expst")
                        nc.scalar.activation(
                            expst[:], psST[:], mybir.ActivationFunctionType.Exp,
                            scale=SCALE,
                        )
                        nc.tensor.matmul(
                            psU[:], V_sb[:, u, h, :], expst[:],
                            start=(n == 0), stop=(n == len(jts) - 1),
                        )
                    nc.vector.tensor_copy(UT_sb[h][:, icols], psU[:])

                    # ---- C4: S -> exp(+accum) -> normalize -> HBM ----
                    for il in range(4):
                        t = ib * 4 + il
                        nvb = (t // 4 + 1) if mode == "causal" else NB
                        P = ppool.tile([128, S], F32, name="P")
                        racc = rpool.tile([128, 4], F32, name="racc")
                        for jb in range(nvb):
                            dg = mode == "causal" and jb == t // 4
                            psS = ps512.tile([128, 512], F32, name="psST")
                            nc.tensor.matmul(
                                psS[:],
                                head_q(h, slice(t * 128, t * 128 + 128)),
                                head_k(h, slice(jb * 512, jb * 512 + 512)),
                                start=True,
                                stop=not (dg or mode == "general"),
                            )
                            if dg:
                                nc.tensor.matmul(
                                    psS[:], ident_sb[:], patt1_sb[:, t % 4, :],
                                    start=False, stop=True,
                                )
                            elif mode == "general":
                                mt = mbst.tile([128, 512], BF16, name="mt")
                                nc.sync.dma_start(
                                    mt[:],
                                    mb1[
                                        t * 128 : t * 128 + 128,
                                        jb * 512 : jb * 512 + 512,
                                    ],
                                )
                                nc.tensor.matmul(
                                    psS[:], ident_sb[:], mt[:], start=False, stop=True
                                )
                            nc.scalar.activation(
                                P[:, jb * 512 : jb * 512 + 512], psS[:],
                                mybir.ActivationFunctionType.Exp, scale=SCALE,
                                accum_out=racc[:, jb : jb + 1],
                            )
                        rsum = rpool.tile([128, 1], F32, name="rsum")
                        nc.vector.tensor_reduce(
                            rsum[:], racc[:, 0:nvb],
                            axis=mybir.AxisListType.X, op=mybir.AluOpType.add,
                        )
                        rinv = rinvT_sb[:, t, h : h + 1]
                        nc.vector.reciprocal(rinv, rsum[:])
                        w = nvb * 512
                        nc.vector.tensor_scalar_mul(P[:, 0:w], P[:, 0:w], rinv)
                        nc.sync.dma_start(
                            attn_out[h, t * 128 : t * 128 + 128, 0:w], P[:, 0:w]
                        )

                # ---- D: output projection for this i-block ----
                # out rows must be normalized per head: (U_h/r_h) @ Wo_h
                # = (U_h @ Wo_h) * rinv_h, applied while accumulating heads.
                for il in range(4):
                    t = ib * 4 + il
                    ob = obuf.tile([128, D_MODEL], F32, name="ob")
                    for et in range(2):
                        oslice = ob[:, et * 512 : et * 512 + 512]
                        for h in range(HPC):
                            psOP = psOPp.tile([128, 512], F32, name="psOP")
                            nc.tensor.matmul(
                                psOP[:],
                                UT_sb[h][:, t * 128 : t * 128 + 128],
                                wo_sb[:, h, et * 512 : et * 512 + 512],
                                start=True, stop=True,
                            )
                            rinv = rinvT_sb[:, t, h : h + 1]
                            if h == 0:
                                nc.vector.tensor_scalar_mul(oslice, psOP[:], rinv)
                            else:
                                nc.vector.scalar_tensor_tensor(
                                    oslice, psOP[:], rinv, oslice,
                                    op0=mybir.AluOpType.mult,
                                    op1=mybir.AluOpType.add,
                                )
                    nc.sync.dma_start(out_p[t * 128 : t * 128 + 128, :], ob[:])

    _split_excess_waits(nc)
    return nc


def _classify_mask(mask):
    m2 = np.asarray(mask).reshape(S, S)
    if np.all(m2 != 0):
        return "dense"
    if np.array_equal(m2 != 0, np.tril(np.ones((S, S), bool))):
        return "causal"
    return "general"


def _make_patterns():
    ii = np.arange(128)[:, None]
    jj = np.arange(512)[None, :]
    p1 = np.zeros((4, 128, 512), np.float32)
    p2 = np.zeros((4, 128, 512), np.float32)
    for k in range(4):
        p1[k] = np.where(jj > ii + 128 * k, -1e30, 0.0)
        # layout-2 tile: partition=j (128), free=i (512)
        p2[k] = np.where(ii + 128 * k > jj, -1e30, 0.0)
    return (
        p1.astype(ml_dtypes.bfloat16),
        p2.astype(ml_dtypes.bfloat16),
    )


def kernel(q, k, v, mask, W_q, b_q, W_k, b_k, W_v, b_v, W_o, b_o):
    q = np.ascontiguousarray(np.asarray(q, np.float32))
    k = np.ascontiguousarray(np.asarray(k, np.float32))
    v = np.ascontiguousarray(np.asarray(v, np.float32))
    W_q = np.asarray(W_q, np.float32)
    W_k = np.asarray(W_k, np.float32)
    W_v = np.asarray(W_v, np.float32)
    W_o = np.asarray(W_o, np.float32)
    b_q = np.asarray(b_q, np.float32)
    b_k = np.asarray(b_k, np.float32)
    b_v = np.asarray(b_v, np.float32)
    b_o = np.asarray(b_o, np.float32)

    mode = _classify_mask(mask)
    nc = _build(mode)

    ident = np.eye(128, dtype=ml_dtypes.bfloat16)
    if mode == "causal":
        patt1, patt2 = _make_patterns()
    elif mode == "general":
        m2 = np.asarray(mask).reshape(S, S)
        mb1 = np.where(m2 != 0, 0.0, -1e30).astype(ml_dtypes.bfloat16)
        mb2 = np.ascontiguousarray(mb1.T)

    in_maps = []
    for c in range(N_CORES):
        bb, hg = c // 4, c % 4
        cols = slice(hg * C_PC, hg * C_PC + C_PC)
        im = {
            "qT": np.ascontiguousarray(q[bb].T),
            "kT": np.ascontiguousarray(k[bb].T),
            "vT": np.ascontiguousarray(v[bb].T),
            "wqT": np.ascontiguousarray(W_q[cols, :].T),
            "wkT": np.ascontiguousarray(W_k[cols, :].T),
            "wvT": np.ascontiguousarray(W_v[cols, :].T),
            "woT": np.ascontiguousarray(W_o[:, cols].T),
            "bq": np.ascontiguousarray(b_q[cols]).reshape(C_PC, 1),
            "bk": np.ascontiguousarray(b_k[cols]).reshape(C_PC, 1),
            "ident": ident,
        }
        if mode == "causal":
            im["patt1"] = patt1
            im["patt2"] = patt2
        elif mode == "general":
            im["mb1"] = mb1
            im["mb2"] = mb2
        in_maps.append(im)

    kw = {}
    if TRACE and _install_ntff_hook():
        kw = dict(trace=True, trace_cores=[0])
    res = run_bass_kernel_spmd(nc, in_maps, core_ids=list(range(N_CORES)), **kw)
    if TRACE:
        _TRACE_RESULT["res"] = res

    attn = np.empty((B, NUM_HEADS, S, S), np.float32)
    out = np.empty((B, S, D_MODEL), np.float32)
    bias_vec = (W_o @ b_v + b_o).astype(np.float32)
    for bb in range(B):
        acc = None
        for hg in range(4):
            r = res.results[bb * 4 + hg]
            attn[bb, hg * HPC : hg * HPC + HPC] = r["attn_out"]
            acc = r["out_p"] if acc is None else acc + r["out_p"]
        out[bb] = acc + bias_vec
    return out, attn


# revision 16
# speedup vs baseline: 1.0719x; 1.0719x over previous
"""Multi-head attention (B=2, S=2048, D=1024, H=16) on 8 trn2 NeuronCores.

Sharding: core c handles batch (c // 4) and heads 4*(c % 4) .. +4 (tensor
parallel over heads, data parallel over batch). Each core computes its 4
heads' Q/K/V projections, the full attention-weight matrix for those heads
(written to HBM as output), and its partial output projection (row-sharded
W_o); the cross-core reduction ("all-reduce after W_o") happens on the host
at unshard time, where the b_v/b_o bias terms are also folded in (exact:
per-row softmax normalization commutes with the W_o contraction, and the
V-bias contributes (W_o @ b_v) to every output row).

Matmuls run in float32r (single-pass fp32, ~1.2e-4 rounding) which streams
at 1 cycle/row for free dims >= 256 vs 4 cycles/row for exact fp32.

Causal masking is done on the tensor engine: an identity x pattern-tile
matmul accumulates -1e30 into masked PSUM score entries before exp (exp
then underflows to exactly 0). Fully masked tiles are skipped and never
written: ExternalOutput buffers are pre-zeroed by the run contract.
"""

import math
from contextlib import ExitStack

import ml_dtypes
import numpy as np

import concourse.bass as bass
import concourse.mybir as mybir
import concourse.tile as tile
from concourse.bass_utils import run_bass_kernel_spmd

D_MODEL = 1024
NUM_HEADS = 16
D_K = 64
B = 2
S = 2048
N_CORES = 8
HPC = 4  # heads per core
C_PC = HPC * D_K  # 256 projected channels per core
SCALE = 1.0 / math.sqrt(D_K)

NB = S // 512  # 4 column blocks of 512
NT = S // 128  # 16 row tiles of 128
KT = D_MODEL // 128  # 8 contraction tiles

F32 = mybir.dt.float32
F32R = mybir.dt.float32r
BF16 = mybir.dt.bfloat16

TRACE = False  # set by test.py for profiling runs
_TRACE_RESULT = {}


def _install_ntff_hook():
    """This image's antenv lacks axon_hooks; synthesize it from the PJRT
    .so's profiling C ABI so run_bass_kernel_spmd's trace path works."""
    import contextlib
    import ctypes
    import os
    import sys
    import types

    try:
        from antenv.axon_hooks import get_axon_ntff_profile_hook  # noqa: F401

        return True
    except ImportError:
        pass
    so = "/opt/axon/libaxon_pjrt.so"
    if not os.path.exists(so):
        return False
    lib = ctypes.CDLL(so)
    if not hasattr(lib, "axon_start_nrt_profile"):
        return False
    lib.axon_start_nrt_profile.argtypes = [
        ctypes.POINTER(ctypes.c_int64),
        ctypes.c_size_t,
    ]
    lib.axon_start_nrt_profile.restype = ctypes.c_int64
    lib.axon_stop_nrt_profile.argtypes = [ctypes.c_char_p]
    lib.axon_stop_nrt_profile.restype = ctypes.c_int64

    @contextlib.contextmanager
    def _hook(output_dir, device_ids):
        import jax

        jax.devices()
        if device_ids:
            ids = (ctypes.c_int64 * len(device_ids))(*device_ids)
            rc = lib.axon_start_nrt_profile(ids, len(device_ids))
        else:
            rc = lib.axon_start_nrt_profile(None, 0)
        if rc != 0:
            raise RuntimeError(f"axon_start_nrt_profile rc={rc}")
        try:
            yield
        finally:
            n = lib.axon_stop_nrt_profile(str(output_dir).encode())
            print(f"ntff profile: {n} file(s) -> {output_dir}", flush=True)

    mod = types.ModuleType("antenv.axon_hooks")
    mod.get_axon_ntff_profile_hook = lambda: _hook
    mod.set_axon_ntff_profile_hook = lambda h: None
    import antenv

    antenv.axon_hooks = mod
    sys.modules["antenv.axon_hooks"] = mod

    # zero-egress container: don't try to upload trace artifacts
    import concourse.bass_utils as bu

    bu.upload_artifacts = lambda tmpdir: f"local://{tmpdir}"
    return True

MM_DT = BF16  # matmul operand dtype: BF16 (1 cyc/row) or F32R (~2.6 cyc/row)
_NP_DT = {BF16: ml_dtypes.bfloat16, F32R: np.float32, F32: np.float32}


def _split_excess_waits(nc, max_waits=1):
    """walrus in this toolchain rejects instructions with more than one
    sync-wait (f32/f32r matmuls fail at 2; the Tile tail drain at 6).
    Move excess waits onto no-fuse NOPs just before the instruction on the
    same engine stream; per-engine order is preserved so this is exact."""
    for f in nc.m.functions:
        for blk in f.blocks:
            insts = blk.instructions
            out = []
            dirty = False
            for inst in insts:
                si = inst.sync_info
                if si is not None and len(si.on_wait) > max_waits:
                    waits = list(si.on_wait)
                    excess, kept = waits[:-max_waits], waits[-max_waits:]
                    for k in range(0, len(excess), max_waits):
                        nop = mybir.InstNoOp(name=f"I-{nc.next_id()}", ins=[], outs=[])
                        nop.engine = inst.engine
                        nop.bass_nofuse = True
                        nop.text_hint = "waitsplit"
                        nop.sync_info = mybir.SyncInfo(
                            on_wait=excess[k : k + max_waits], on_update=[]
                        )
                        nc.register_instruction(nop, overwrite=True)
                        out.append(nop)
                    inst.sync_info = mybir.SyncInfo(
                        on_wait=kept, on_update=list(si.on_update)
                    )
                    dirty = True
                out.append(inst)
            if dirty:
                blk.instructions = out


def _valid_jtiles(ib, mode):
    """128-wide key tiles that intersect the unmasked region for query block
    ib (512 queries)."""
    if mode == "causal":
        return list(range(4 * ib + 4))
    return list(range(NT))


def _build(mode):
    """mode: 'causal' | 'dense' | 'general'."""
    nc = bass.Bass("TRN2", target_bir_lowering=False, debug=False)

    qT = nc.dram_tensor("qT", [D_MODEL, S], MM_DT, kind="ExternalInput").ap()
    kT = nc.dram_tensor("kT", [D_MODEL, S], MM_DT, kind="ExternalInput").ap()
    vT = nc.dram_tensor("vT", [D_MODEL, S], MM_DT, kind="ExternalInput").ap()
    wqT = nc.dram_tensor("wqT", [D_MODEL, C_PC], MM_DT, kind="ExternalInput").ap()
    wkT = nc.dram_tensor("wkT", [D_MODEL, C_PC], MM_DT, kind="ExternalInput").ap()
    wvT = nc.dram_tensor("wvT", [D_MODEL, C_PC], MM_DT, kind="ExternalInput").ap()
    woT = nc.dram_tensor("woT", [C_PC, D_MODEL], MM_DT, kind="ExternalInput").ap()
    bq = nc.dram_tensor("bq", [C_PC, 1], F32, kind="ExternalInput").ap()
    bk = nc.dram_tensor("bk", [C_PC, 1], F32, kind="ExternalInput").ap()
    ident = nc.dram_tensor("ident", [128, 128], BF16, kind="ExternalInput").ap()
    if mode == "causal":
        patt1 = nc.dram_tensor("patt1", [4, 128, 512], BF16, kind="ExternalInput").ap()
        patt2 = nc.dram_tensor("patt2", [4, 128, 512], BF16, kind="ExternalInput").ap()
    elif mode == "general":
        mb1 = nc.dram_tensor("mb1", [S, S], BF16, kind="ExternalInput").ap()
        mb2 = nc.dram_tensor("mb2", [S, S], BF16, kind="ExternalInput").ap()

    attn_out = nc.dram_tensor("attn_out", [HPC, S, S], F32, kind="ExternalOutput").ap()
    out_p = nc.dram_tensor("out_p", [S, D_MODEL], F32, kind="ExternalOutput").ap()

    with tile.TileContext(nc) as tc, ExitStack() as ctx:
        const = ctx.enter_context(tc.tile_pool(name="const", bufs=1))
        resident = ctx.enter_context(tc.tile_pool(name="resident", bufs=1))

        # --- constants / weights ---
        wq_sb = const.tile([128, KT, C_PC], MM_DT)
        wk_sb = const.tile([128, KT, C_PC], MM_DT)
        wv_sb = const.tile([128, KT, C_PC], MM_DT)
        nc.sync.dma_start(wq_sb[:], wqT.rearrange("(kt p) c -> p kt c", p=128))
        nc.sync.dma_start(wk_sb[:], wkT.rearrange("(kt p) c -> p kt c", p=128))
        nc.sync.dma_start(wv_sb[:], wvT.rearrange("(kt p) c -> p kt c", p=128))
        # head-pair packed: partitions 0-63 = even head, 64-127 = odd head
        wo_sb = const.tile([128, 2, D_MODEL], MM_DT)
        nc.sync.dma_start(wo_sb[:], woT.rearrange("(g p) e -> p g e", p=128))
        bq_sb = const.tile([128, 2], F32)
        bk_sb = const.tile([128, 2], F32)
        nc.sync.dma_start(bq_sb[:], bq.rearrange("(m p) one -> p (m one)", p=128))
        nc.sync.dma_start(bk_sb[:], bk.rearrange("(m p) one -> p (m one)", p=128))
        ident_sb = const.tile([128, 128], BF16)
        nc.sync.dma_start(ident_sb[:], ident)
        if mode == "causal":
            patt1_sb = const.tile([128, 4, 512], BF16)
            patt2_sb = const.tile([128, 4, 512], BF16)
            nc.sync.dma_start(patt1_sb[:], patt1.rearrange("k p j -> p k j"))
            nc.sync.dma_start(patt2_sb[:], patt2.rearrange("k p j -> p k j"))

        # --- resident activations ---
        QT_sb = [resident.tile([128, S], MM_DT, name=f"QT{m}") for m in range(2)]
        KT_sb = [resident.tile([128, S], MM_DT, name=f"KT{m}") for m in range(2)]
        V_sb = resident.tile([128, NT, HPC, D_K], MM_DT)
        # unnormalized U^T, head-pair packed on partitions
        UT_sb = [resident.tile([128, S], MM_DT, name=f"UTp{g}") for g in range(2)]
        # 1/rowsum per (query, i-tile, head), partition layout, from C4
        rinvT_sb = resident.tile([128, NT, HPC], F32)

        def head_q(h, cols):  # [64, w] slice of Q^T for head h
            return QT_sb[h // 2][(h % 2) * 64 : (h % 2) * 64 + 64, cols]

        def head_k(h, cols):
            return KT_sb[h // 2][(h % 2) * 64 : (h % 2) * 64 + 64, cols]

        # ================= phase B: projections =================
        with (
            tc.tile_pool(name="stage", bufs=2) as stage,
            tc.tile_pool(name="psB", bufs=1, space="PSUM") as psB,
        ):
            for nb in range(NB):
                cols = slice(nb * 512, nb * 512 + 512)
                for quarter in range(4):  # kt in four pairs
                    kts = range(quarter * 2, quarter * 2 + 2)
                    q_st = stage.tile([128, 2, 512], MM_DT, name="q_st")
                    k_st = stage.tile([128, 2, 512], MM_DT, name="k_st")
                    v_st = stage.tile([128, 2, 512], MM_DT, name="v_st")
                    src = qT.rearrange("(kt p) s -> p kt s", p=128)
                    nc.sync.dma_start(q_st[:], src[:, quarter * 2 : quarter * 2 + 2, cols])
                    src = kT.rearrange("(kt p) s -> p kt s", p=128)
                    nc.sync.dma_start(k_st[:], src[:, quarter * 2 : quarter * 2 + 2, cols])
                    src = vT.rearrange("(kt p) s -> p kt s", p=128)
                    nc.sync.dma_start(v_st[:], src[:, quarter * 2 : quarter * 2 + 2, cols])
                    if quarter == 0:
                        psQ = [
                            psB.tile([128, 512], F32, name="psQ", bufs=2)
                            for _ in range(2)
                        ]
                        psK = [
                            psB.tile([128, 512], F32, name="psK", bufs=2)
                            for _ in range(2)
                        ]
                        psV = [
                            psB.tile([128, 256], F32, name="psV", bufs=4)
                            for _ in range(4)
                        ]
                    for i, kt in enumerate(kts):
                        first = kt == 0
                        last = kt == KT - 1
                        for m in range(2):
                            mc = slice(m * 128, m * 128 + 128)
                            nc.tensor.matmul(
                                psQ[m][:], wq_sb[:, kt, mc], q_st[:, i, :],
                                start=first, stop=last,
                            )
                            nc.tensor.matmul(
                                psK[m][:], wk_sb[:, kt, mc], k_st[:, i, :],
                                start=first, stop=last,
                            )
                        for jl in range(4):
                            nc.tensor.matmul(
                                psV[jl][:],
                                v_st[:, i, jl * 128 : jl * 128 + 128],
                                wv_sb[:, kt, :],
                                start=first, stop=last,
                            )
                for m in range(2):
                    nc.scalar.add(QT_sb[m][:, cols], psQ[m][:], bq_sb[:, m : m + 1])
                    nc.scalar.add(KT_sb[m][:, cols], psK[m][:], bk_sb[:, m : m + 1])
                for jl in range(4):
                    jt = nb * 4 + jl
                    nc.vector.tensor_copy(
                        V_sb[:, jt, :, 0:D_K],
                        psV[jl][:].rearrange("p (h c) -> p h c", h=HPC),
                    )

        # ================= phases C & D =================
        with (
            tc.tile_pool(name="ps512", bufs=4, space="PSUM") as ps512,
            tc.tile_pool(name="psU", bufs=2, space="PSUM") as psUp,
            tc.tile_pool(name="psOP", bufs=2, space="PSUM") as psOPp,
            tc.tile_pool(name="expp", bufs=4) as expp,
            tc.tile_pool(name="ppool", bufs=2) as ppool,
            tc.tile_pool(name="rpool", bufs=8) as rpool,
            tc.tile_pool(name="mbst", bufs=4) as mbst,
            tc.tile_pool(name="obuf", bufs=2) as obuf,
        ):
            def masked_scores(psS, lhsT, rhs, diag_patt, gen_rows, gen_cols):
                """scores matmul + mask accumulation into the same PSUM tile."""
                plain = diag_patt is None and mode != "general"
                nc.tensor.matmul(psS, lhsT, rhs, start=True, stop=plain)
                if diag_patt is not None:
                    nc.tensor.matmul(psS, ident_sb[:], diag_patt, start=False, stop=True)
                elif mode == "general":
                    mt = mbst.tile([128, 512], BF16, name="mt")
                    src = mb1 if gen_rows is not None else mb2
                    nc.sync.dma_start(mt[:], src[gen_rows or gen_cols])
                    nc.tensor.matmul(psS, ident_sb[:], mt[:], start=False, stop=True)

            for ib in range(NB):
                icols = slice(ib * 512, ib * 512 + 512)
                for g in range(2):  # head pairs (2g, 2g+1) -> PE row groups
                    pair = (2 * g, 2 * g + 1)
                    # ---- C1/C2: S^T -> exp -> U^T (unnormalized) ----
                    jts = _valid_jtiles(ib, mode)
                    psU = psUp.tile([128, 512], F32, name="psU")
                    for n, u in enumerate(jts):
                        diag = mode == "causal" and u >= 4 * ib
                        for h in pair:
                            psST = ps512.tile([128, 512], F32, name="psST")
                            masked_scores(
                                psST[:],
                                head_k(h, slice(u * 128, u * 128 + 128)),
                                head_q(h, icols),
                                patt2_sb[:, u - 4 * ib, :] if diag else None,
                                None,
                                (slice(u * 128, u * 128 + 128), icols),
                            )
                            expst = expp.tile([128, 512], MM_DT, name="expst")
                            nc.scalar.activation(
                                expst[:], psST[:], mybir.ActivationFunctionType.Exp,
                                scale=SCALE,
                            )
                            # odd head lands in PSUM partitions 64-127 (col group)
                            po = (h % 2) * 64
                            nc.tensor.matmul(
                                psU[po : po + 64, :], V_sb[:, u, h, :], expst[:],
                                start=(n == 0), stop=(n == len(jts) - 1),
                            )
                    nc.vector.tensor_copy(UT_sb[g][:, icols], psU[:])

                    # ---- C4: S -> exp(+accum) -> normalize -> HBM ----
                    for il in range(4):
                        t = ib * 4 + il
                        nvb = (t // 4 + 1) if mode == "causal" else NB
                        Ps = {}
                        raccs = {}
                        for h in pair:
                            Ps[h] = ppool.tile([128, S], F32, name="P")
                            raccs[h] = rpool.tile([128, 4], F32, name="racc")
                        for jb in range(nvb):
                            dg = mode == "causal" and jb == t // 4
                            for h in pair:
                                psS = ps512.tile([128, 512], F32, name="psST")
                                masked_scores(
                                    psS[:],
                                    head_q(h, slice(t * 128, t * 128 + 128)),
                                    head_k(h, slice(jb * 512, jb * 512 + 512)),
                                    patt1_sb[:, t % 4, :] if dg else None,
                                    (
                                        slice(t * 128, t * 128 + 128),
                                        slice(jb * 512, jb * 512 + 512),
                                    ),
                                    None,
                                )
                                nc.scalar.activation(
                                    Ps[h][:, jb * 512 : jb * 512 + 512], psS[:],
                                    mybir.ActivationFunctionType.Exp, scale=SCALE,
                                    accum_out=raccs[h][:, jb : jb + 1],
                                )
                        for h in pair:
                            rsum = rpool.tile([128, 1], F32, name="rsum")
                            nc.vector.tensor_reduce(
                                rsum[:], raccs[h][:, 0:nvb],
                                axis=mybir.AxisListType.X, op=mybir.AluOpType.add,
                            )
                            rinv = rinvT_sb[:, t, h : h + 1]
                            nc.vector.reciprocal(rinv, rsum[:])
                            w = nvb * 512
                            nc.vector.tensor_scalar_mul(
                                Ps[h][:, 0:w], Ps[h][:, 0:w], rinv
                            )
                            nc.sync.dma_start(
                                attn_out[h, t * 128 : t * 128 + 128, 0:w],
                                Ps[h][:, 0:w],
                            )

                # ---- D: output projection for this i-block ----
                # out rows must be normalized per head: (U_h/r_h) @ Wo_h
                # = (U_h @ Wo_h) * rinv_h, applied while accumulating heads.
                for il in range(4):
                    t = ib * 4 + il
                    ob = obuf.tile([128, D_MODEL], F32, name="ob")
                    for et in range(2):
                        oslice = ob[:, et * 512 : et * 512 + 512]
                        psOPs = {}
                        for h in range(HPC):  # row-group pairs run concurrently
                            po = (h % 2) * 64
                            psOPs[h] = psOPp.tile([128, 512], F32, name="psOP")
                            nc.tensor.matmul(
                                psOPs[h][:],
                                UT_sb[h // 2][po : po + 64, t * 128 : t * 128 + 128],
                                wo_sb[po : po + 64, h // 2, et * 512 : et * 512 + 512],
                                start=True, stop=True,
                            )
                        for h in range(HPC):
                            rinv = rinvT_sb[:, t, h : h + 1]
                            if h == 0:
                                nc.vector.tensor_scalar_mul(oslice, psOPs[h][:], rinv)
                            else:
                                nc.vector.scalar_tensor_tensor(
                                    oslice, psOPs[h][:], rinv, oslice,
                                    op0=mybir.AluOpType.mult,
                                    op1=mybir.AluOpType.add,
                                )
                    nc.sync.dma_start(out_p[t * 128 : t * 128 + 128, :], ob[:])

    _split_excess_waits(nc)
    return nc


def _classify_mask(mask):
    m2 = np.asarray(mask).reshape(S, S)
    if np.all(m2 != 0):
        return "dense"
    if np.array_equal(m2 != 0, np.tril(np.ones((S, S), bool))):
        return "causal"
    return "general"


def _make_patterns():
    ii = np.arange(128)[:, None]
    jj = np.arange(512)[None, :]
    p1 = np.zeros((4, 128, 512), np.float32)
    p2 = np.zeros((4, 128, 512), np.float32)
    for k in range(4):
        p1[k] = np.where(jj > ii + 128 * k, -1e30, 0.0)
        # layout-2 tile: partition=j (128), free=i (512)
        p2[k] = np.where(ii + 128 * k > jj, -1e30, 0.0)
    return (
        p1.astype(ml_dtypes.bfloat16),
        p2.astype(ml_dtypes.bfloat16),
    )


def kernel(q, k, v, mask, W_q, b_q, W_k, b_k, W_v, b_v, W_o, b_o):
    q = np.ascontiguousarray(np.asarray(q, np.float32))
    k = np.ascontiguousarray(np.asarray(k, np.float32))
    v = np.ascontiguousarray(np.asarray(v, np.float32))
    W_q = np.asarray(W_q, np.float32)
    W_k = np.asarray(W_k, np.float32)
    W_v = np.asarray(W_v, np.float32)
    W_o = np.asarray(W_o, np.float32)
    b_q = np.asarray(b_q, np.float32)
    b_k = np.asarray(b_k, np.float32)
    b_v = np.asarray(b_v, np.float32)
    b_o = np.asarray(b_o, np.float32)

    mode = _classify_mask(mask)
    nc = _build(mode)

    ident = np.eye(128, dtype=ml_dtypes.bfloat16)
    if mode == "causal":
        patt1, patt2 = _make_patterns()
    elif mode == "general":
        m2 = np.asarray(mask).reshape(S, S)
        mb1 = np.where(m2 != 0, 0.0, -1e30).astype(ml_dtypes.bfloat16)
        mb2 = np.ascontiguousarray(mb1.T)

    dt = _NP_DT[MM_DT]
    in_maps = []
    for c in range(N_CORES):
        bb, hg = c // 4, c % 4
        cols = slice(hg * C_PC, hg * C_PC + C_PC)
        im = {
            "qT": np.ascontiguousarray(q[bb].T).astype(dt),
            "kT": np.ascontiguousarray(k[bb].T).astype(dt),
            "vT": np.ascontiguousarray(v[bb].T).astype(dt),
            "wqT": np.ascontiguousarray(W_q[cols, :].T).astype(dt),
            "wkT": np.ascontiguousarray(W_k[cols, :].T).astype(dt),
            "wvT": np.ascontiguousarray(W_v[cols, :].T).astype(dt),
            "woT": np.ascontiguousarray(W_o[:, cols].T).astype(dt),
            "bq": np.ascontiguousarray(b_q[cols]).reshape(C_PC, 1),
            "bk": np.ascontiguousarray(b_k[cols]).reshape(C_PC, 1),
            "ident": ident,
        }
        if mode == "causal":
            im["patt1"] = patt1
            im["patt2"] = patt2
        elif mode == "general":
            im["mb1"] = mb1
            im["mb2"] = mb2
        in_maps.append(im)

    kw = {}
    if TRACE and _install_ntff_hook():
        kw = dict(trace=True, trace_cores=[0])
    res = run_bass_kernel_spmd(nc, in_maps, core_ids=list(range(N_CORES)), **kw)
    if TRACE:
        _TRACE_RESULT["res"] = res

    attn = np.empty((B, NUM_HEADS, S, S), np.float32)
    out = np.empty((B, S, D_MODEL), np.float32)
    bias_vec = (W_o @ b_v + b_o).astype(np.float32)
    for bb in range(B):
        acc = None
        for hg in range(4):
            r = res.results[bb * 4 + hg]
            attn[bb, hg * HPC : hg * HPC + HPC] = r["attn_out"]
            acc = r["out_p"] if acc is None else acc + r["out_p"]
        out[bb] = acc + bias_vec
    return out, attn


# revision 18
# speedup vs baseline: 1.2879x; 1.2016x over previous
"""Multi-head attention (B=2, S=2048, D=1024, H=16) on 8 trn2 NeuronCores.

Sharding: core c handles batch (c // 4) and heads 4*(c % 4) .. +4 (tensor
parallel over heads, data parallel over batch). Each core computes its 4
heads' Q/K/V projections, the full attention-weight matrix for those heads
(written to HBM as output), and its partial output projection (row-sharded
W_o); the cross-core reduction ("all-reduce after W_o") happens on the host
at unshard time, where the b_v/b_o bias terms are also folded in (exact:
per-row softmax normalization commutes with the W_o contraction, and the
V-bias contributes (W_o @ b_v) to every output row).

Matmuls run in float32r (single-pass fp32, ~1.2e-4 rounding) which streams
at 1 cycle/row for free dims >= 256 vs 4 cycles/row for exact fp32.

Causal masking is done on the tensor engine: an identity x pattern-tile
matmul accumulates -1e30 into masked PSUM score entries before exp (exp
then underflows to exactly 0). Fully masked tiles are skipped and never
written: ExternalOutput buffers are pre-zeroed by the run contract.
"""

import math
from contextlib import ExitStack

import ml_dtypes
import numpy as np

import concourse.bass as bass
import concourse.mybir as mybir
import concourse.tile as tile
from concourse.bass_utils import run_bass_kernel_spmd

D_MODEL = 1024
NUM_HEADS = 16
D_K = 64
B = 2
S = 2048
N_CORES = 8
HPC = 4  # heads per core
C_PC = HPC * D_K  # 256 projected channels per core
SCALE = 1.0 / math.sqrt(D_K)

NB = S // 512  # 4 column blocks of 512
NT = S // 128  # 16 row tiles of 128
KT = D_MODEL // 128  # 8 contraction tiles

F32 = mybir.dt.float32
F32R = mybir.dt.float32r
BF16 = mybir.dt.bfloat16

TRACE = False  # set by test.py for profiling runs
_TRACE_RESULT = {}


def _install_ntff_hook():
    """This image's antenv lacks axon_hooks; synthesize it from the PJRT
    .so's profiling C ABI so run_bass_kernel_spmd's trace path works."""
    import contextlib
    import ctypes
    import os
    import sys
    import types

    try:
        from antenv.axon_hooks import get_axon_ntff_profile_hook  # noqa: F401

        return True
    except ImportError:
        pass
    so = "/opt/axon/libaxon_pjrt.so"
    if not os.path.exists(so):
        return False
    lib = ctypes.CDLL(so)
    if not hasattr(lib, "axon_start_nrt_profile"):
        return False
    lib.axon_start_nrt_profile.argtypes = [
        ctypes.POINTER(ctypes.c_int64),
        ctypes.c_size_t,
    ]
    lib.axon_start_nrt_profile.restype = ctypes.c_int64
    lib.axon_stop_nrt_profile.argtypes = [ctypes.c_char_p]
    lib.axon_stop_nrt_profile.restype = ctypes.c_int64

    @contextlib.contextmanager
    def _hook(output_dir, device_ids):
        import jax

        jax.devices()
        if device_ids:
            ids = (ctypes.c_int64 * len(device_ids))(*device_ids)
            rc = lib.axon_start_nrt_profile(ids, len(device_ids))
        else:
            rc = lib.axon_start_nrt_profile(None, 0)
        if rc != 0:
            raise RuntimeError(f"axon_start_nrt_profile rc={rc}")
        try:
            yield
        finally:
            n = lib.axon_stop_nrt_profile(str(output_dir).encode())
            print(f"ntff profile: {n} file(s) -> {output_dir}", flush=True)

    mod = types.ModuleType("antenv.axon_hooks")
    mod.get_axon_ntff_profile_hook = lambda: _hook
    mod.set_axon_ntff_profile_hook = lambda h: None
    import antenv

    antenv.axon_hooks = mod
    sys.modules["antenv.axon_hooks"] = mod

    # zero-egress container: don't try to upload trace artifacts
    import concourse.bass_utils as bu

    bu.upload_artifacts = lambda tmpdir: f"local://{tmpdir}"
    return True

MM_DT = BF16  # matmul operand dtype: BF16 (1 cyc/row) or F32R (~2.6 cyc/row)
_NP_DT = {BF16: ml_dtypes.bfloat16, F32R: np.float32, F32: np.float32}


def _split_excess_waits(nc, max_waits=1):
    """walrus in this toolchain rejects instructions with more than one
    sync-wait (f32/f32r matmuls fail at 2; the Tile tail drain at 6).
    Move excess waits onto no-fuse NOPs just before the instruction on the
    same engine stream; per-engine order is preserved so this is exact."""
    for f in nc.m.functions:
        for blk in f.blocks:
            insts = blk.instructions
            out = []
            dirty = False
            for inst in insts:
                si = inst.sync_info
                if si is not None and len(si.on_wait) > max_waits:
                    waits = list(si.on_wait)
                    excess, kept = waits[:-max_waits], waits[-max_waits:]
                    for k in range(0, len(excess), max_waits):
                        nop = mybir.InstNoOp(name=f"I-{nc.next_id()}", ins=[], outs=[])
                        nop.engine = inst.engine
                        nop.bass_nofuse = True
                        nop.text_hint = "waitsplit"
                        nop.sync_info = mybir.SyncInfo(
                            on_wait=excess[k : k + max_waits], on_update=[]
                        )
                        nc.register_instruction(nop, overwrite=True)
                        out.append(nop)
                    inst.sync_info = mybir.SyncInfo(
                        on_wait=kept, on_update=list(si.on_update)
                    )
                    dirty = True
                out.append(inst)
            if dirty:
                blk.instructions = out


def _valid_jtiles(ib, mode):
    """128-wide key tiles that intersect the unmasked region for query block
    ib (512 queries)."""
    if mode == "causal":
        return list(range(4 * ib + 4))
    return list(range(NT))


def _build(mode):
    """mode: 'causal' | 'dense' | 'general'."""
    nc = bass.Bass("TRN2", target_bir_lowering=False, debug=False)

    qT = nc.dram_tensor("qT", [D_MODEL, S], MM_DT, kind="ExternalInput").ap()
    kT = nc.dram_tensor("kT", [D_MODEL, S], MM_DT, kind="ExternalInput").ap()
    vT = nc.dram_tensor("vT", [D_MODEL, S], MM_DT, kind="ExternalInput").ap()
    wqT = nc.dram_tensor("wqT", [D_MODEL, C_PC], MM_DT, kind="ExternalInput").ap()
    wkT = nc.dram_tensor("wkT", [D_MODEL, C_PC], MM_DT, kind="ExternalInput").ap()
    wvT = nc.dram_tensor("wvT", [D_MODEL, C_PC], MM_DT, kind="ExternalInput").ap()
    woT = nc.dram_tensor("woT", [C_PC, D_MODEL], MM_DT, kind="ExternalInput").ap()
    bq = nc.dram_tensor("bq", [C_PC, 1], F32, kind="ExternalInput").ap()
    bk = nc.dram_tensor("bk", [C_PC, 1], F32, kind="ExternalInput").ap()
    ident = nc.dram_tensor("ident", [128, 128], BF16, kind="ExternalInput").ap()
    if mode == "causal":
        patt1 = nc.dram_tensor("patt1", [4, 128, 512], BF16, kind="ExternalInput").ap()
        patt2 = nc.dram_tensor("patt2", [4, 128, 512], BF16, kind="ExternalInput").ap()
    elif mode == "general":
        mb1 = nc.dram_tensor("mb1", [S, S], BF16, kind="ExternalInput").ap()
        mb2 = nc.dram_tensor("mb2", [S, S], BF16, kind="ExternalInput").ap()

    attn_out = nc.dram_tensor("attn_out", [HPC, S, S], F32, kind="ExternalOutput").ap()
    out_p = nc.dram_tensor("out_p", [S, D_MODEL], F32, kind="ExternalOutput").ap()

    with tile.TileContext(nc) as tc, ExitStack() as ctx:
        const = ctx.enter_context(tc.tile_pool(name="const", bufs=1))
        resident = ctx.enter_context(tc.tile_pool(name="resident", bufs=1))

        # --- constants / weights ---
        wq_sb = const.tile([128, KT, C_PC], MM_DT)
        wk_sb = const.tile([128, KT, C_PC], MM_DT)
        wv_sb = const.tile([128, KT, C_PC], MM_DT)
        nc.sync.dma_start(wq_sb[:], wqT.rearrange("(kt p) c -> p kt c", p=128))
        nc.sync.dma_start(wk_sb[:], wkT.rearrange("(kt p) c -> p kt c", p=128))
        nc.sync.dma_start(wv_sb[:], wvT.rearrange("(kt p) c -> p kt c", p=128))
        # head-pair packed: partitions 0-63 = even head, 64-127 = odd head
        wo_sb = const.tile([128, 2, D_MODEL], MM_DT)
        nc.sync.dma_start(wo_sb[:], woT.rearrange("(g p) e -> p g e", p=128))
        bq_sb = const.tile([128, 2], F32)
        bk_sb = const.tile([128, 2], F32)
        nc.sync.dma_start(bq_sb[:], bq.rearrange("(m p) one -> p (m one)", p=128))
        nc.sync.dma_start(bk_sb[:], bk.rearrange("(m p) one -> p (m one)", p=128))
        ident_sb = const.tile([128, 128], BF16)
        nc.sync.dma_start(ident_sb[:], ident)
        if mode == "causal":
            patt1_sb = const.tile([128, 4, 512], BF16)
            patt2_sb = const.tile([128, 4, 512], BF16)
            nc.sync.dma_start(patt1_sb[:], patt1.rearrange("k p j -> p k j"))
            nc.sync.dma_start(patt2_sb[:], patt2.rearrange("k p j -> p k j"))

        # --- resident activations ---
        QT_sb = [resident.tile([128, S], MM_DT, name=f"QT{m}") for m in range(2)]
        KT_sb = [resident.tile([128, S], MM_DT, name=f"KT{m}") for m in range(2)]
        V_sb = resident.tile([128, NT, HPC, D_K], MM_DT)
        # unnormalized U^T, head-pair packed on partitions
        UT_sb = [resident.tile([128, S], MM_DT, name=f"UTp{g}") for g in range(2)]
        # 1/rowsum per (query, i-tile, head), partition layout, from C4
        rinvT_sb = resident.tile([128, NT, HPC], F32)

        def head_q(h, cols):  # [64, w] slice of Q^T for head h
            return QT_sb[h // 2][(h % 2) * 64 : (h % 2) * 64 + 64, cols]

        def head_k(h, cols):
            return KT_sb[h // 2][(h % 2) * 64 : (h % 2) * 64 + 64, cols]

        # ================= phase B: projections =================
        with (
            tc.tile_pool(name="stage", bufs=2) as stage,
            tc.tile_pool(name="psB", bufs=1, space="PSUM") as psB,
        ):
            for nb in range(NB):
                cols = slice(nb * 512, nb * 512 + 512)
                for quarter in range(4):  # kt in four pairs
                    kts = range(quarter * 2, quarter * 2 + 2)
                    q_st = stage.tile([128, 2, 512], MM_DT, name="q_st")
                    k_st = stage.tile([128, 2, 512], MM_DT, name="k_st")
                    v_st = stage.tile([128, 2, 512], MM_DT, name="v_st")
                    src = qT.rearrange("(kt p) s -> p kt s", p=128)
                    nc.sync.dma_start(q_st[:], src[:, quarter * 2 : quarter * 2 + 2, cols])
                    src = kT.rearrange("(kt p) s -> p kt s", p=128)
                    nc.sync.dma_start(k_st[:], src[:, quarter * 2 : quarter * 2 + 2, cols])
                    src = vT.rearrange("(kt p) s -> p kt s", p=128)
                    nc.sync.dma_start(v_st[:], src[:, quarter * 2 : quarter * 2 + 2, cols])
                    if quarter == 0:
                        psQ = [
                            psB.tile([128, 512], F32, name="psQ", bufs=2)
                            for _ in range(2)
                        ]
                        psK = [
                            psB.tile([128, 512], F32, name="psK", bufs=2)
                            for _ in range(2)
                        ]
                        psV = [
                            psB.tile([128, 256], F32, name="psV", bufs=4)
                            for _ in range(4)
                        ]
                    for i, kt in enumerate(kts):
                        first = kt == 0
                        last = kt == KT - 1
                        for m in range(2):
                            mc = slice(m * 128, m * 128 + 128)
                            nc.tensor.matmul(
                                psQ[m][:], wq_sb[:, kt, mc], q_st[:, i, :],
                                start=first, stop=last,
                            )
                            nc.tensor.matmul(
                                psK[m][:], wk_sb[:, kt, mc], k_st[:, i, :],
                                start=first, stop=last,
                            )
                        for jl in range(4):
                            nc.tensor.matmul(
                                psV[jl][:],
                                v_st[:, i, jl * 128 : jl * 128 + 128],
                                wv_sb[:, kt, :],
                                start=first, stop=last,
                            )
                for m in range(2):
                    nc.scalar.add(QT_sb[m][:, cols], psQ[m][:], bq_sb[:, m : m + 1])
                    nc.scalar.add(KT_sb[m][:, cols], psK[m][:], bk_sb[:, m : m + 1])
                for jl in range(4):
                    jt = nb * 4 + jl
                    nc.vector.tensor_copy(
                        V_sb[:, jt, :, 0:D_K],
                        psV[jl][:].rearrange("p (h c) -> p h c", h=HPC),
                    )

        # ================= phases C & D =================
        with (
            tc.tile_pool(name="ps512", bufs=2, space="PSUM") as ps512,
            tc.tile_pool(name="psU", bufs=2, space="PSUM") as psUp,
            tc.tile_pool(name="psOP", bufs=2, space="PSUM") as psOPp,
            tc.tile_pool(name="expp", bufs=4) as expp,
            tc.tile_pool(name="ppool", bufs=2) as ppool,
            tc.tile_pool(name="rpool", bufs=8) as rpool,
            tc.tile_pool(name="mbst", bufs=4) as mbst,
            tc.tile_pool(name="obuf", bufs=2) as obuf,
        ):
            def masked_scores(psS, lhsT, rhs, diag_patt, gen_rows, gen_cols):
                """scores matmul + mask accumulation into the same PSUM tile."""
                plain = diag_patt is None and mode != "general"
                nc.tensor.matmul(psS, lhsT, rhs, start=True, stop=plain)
                if diag_patt is not None:
                    nc.tensor.matmul(psS, ident_sb[:], diag_patt, start=False, stop=True)
                elif mode == "general":
                    mt = mbst.tile([128, 512], BF16, name="mt")
                    src = mb1 if gen_rows is not None else mb2
                    nc.sync.dma_start(mt[:], src[gen_rows or gen_cols])
                    nc.tensor.matmul(psS, ident_sb[:], mt[:], start=False, stop=True)

            for ib in range(NB):
                icols = slice(ib * 512, ib * 512 + 512)
                for g in range(2):  # head pairs (2g, 2g+1) -> PE row groups
                    pair = (2 * g, 2 * g + 1)
                    # ---- C1/C2: S^T -> exp -> U^T (unnormalized) ----
                    # U matmuls run one step behind exp so the PE never
                    # stalls waiting on the scalar engine.
                    jts = _valid_jtiles(ib, mode)
                    psU = psUp.tile([128, 512], F32, name="psU")
                    last = len(jts) - 1

                    def emit_U(n, u, expst):
                        for h in pair:
                            po = (h % 2) * 64
                            nc.tensor.matmul(
                                psU[po : po + 64, :],
                                V_sb[:, u, h, :],
                                expst[:, (h % 2) * 512 : (h % 2) * 512 + 512],
                                start=(n == 0), stop=(n == last),
                            )

                    prev = None
                    for n, u in enumerate(jts):
                        diag = mode == "causal" and u >= 4 * ib
                        psST = ps512.tile([128, 1024], F32, name="psST")
                        for h in pair:
                            off = (h % 2) * 512
                            masked_scores(
                                psST[:, off : off + 512],
                                head_k(h, slice(u * 128, u * 128 + 128)),
                                head_q(h, icols),
                                patt2_sb[:, u - 4 * ib, :] if diag else None,
                                None,
                                (slice(u * 128, u * 128 + 128), icols),
                            )
                        expst = expp.tile([128, 1024], MM_DT, name="expst")
                        nc.scalar.activation(
                            expst[:], psST[:], mybir.ActivationFunctionType.Exp,
                            scale=SCALE,
                        )
                        if prev is not None:
                            emit_U(*prev)
                        prev = (n, u, expst)
                    emit_U(*prev)
                    nc.vector.tensor_copy(UT_sb[g][:, icols], psU[:])

                    # ---- C4: S -> exp -> rowsum -> normalize -> HBM ----
                    for il in range(4):
                        t = ib * 4 + il
                        nvb = (t // 4 + 1) if mode == "causal" else NB
                        w = nvb * 512
                        Pp = ppool.tile([128, 2, S], F32, name="P")
                        for jb in range(nvb):
                            dg = mode == "causal" and jb == t // 4
                            psS = ps512.tile([128, 1024], F32, name="psST")
                            for h in pair:
                                off = (h % 2) * 512
                                masked_scores(
                                    psS[:, off : off + 512],
                                    head_q(h, slice(t * 128, t * 128 + 128)),
                                    head_k(h, slice(jb * 512, jb * 512 + 512)),
                                    patt1_sb[:, t % 4, :] if dg else None,
                                    (
                                        slice(t * 128, t * 128 + 128),
                                        slice(jb * 512, jb * 512 + 512),
                                    ),
                                    None,
                                )
                            nc.scalar.activation(
                                Pp[:, :, jb * 512 : jb * 512 + 512],
                                psS[:].rearrange("p (h j) -> p h j", h=2),
                                mybir.ActivationFunctionType.Exp, scale=SCALE,
                            )
                        rsum = rpool.tile([128, 2], F32, name="rsum")
                        nc.vector.tensor_reduce(
                            rsum[:], Pp[:, :, 0:w],
                            axis=mybir.AxisListType.X, op=mybir.AluOpType.add,
                        )
                        rinv2 = rinvT_sb[:, t, 2 * g : 2 * g + 2]
                        nc.vector.reciprocal(rinv2, rsum[:])
                        for h in pair:
                            hp = h % 2
                            nc.vector.tensor_scalar_mul(
                                Pp[:, hp, 0:w], Pp[:, hp, 0:w],
                                rinvT_sb[:, t, h : h + 1],
                            )
                            nc.sync.dma_start(
                                attn_out[h, t * 128 : t * 128 + 128, 0:w],
                                Pp[:, hp, 0:w],
                            )

                # ---- D: output projection for this i-block ----
                # out rows must be normalized per head: (U_h/r_h) @ Wo_h
                # = (U_h @ Wo_h) * rinv_h, applied while accumulating heads.
                for il in range(4):
                    t = ib * 4 + il
                    ob = obuf.tile([128, D_MODEL], F32, name="ob")
                    for et in range(2):
                        oslice = ob[:, et * 512 : et * 512 + 512]
                        psOPs = {}
                        for h in range(HPC):  # row-group pairs run concurrently
                            po = (h % 2) * 64
                            psOPs[h] = psOPp.tile([128, 512], F32, name="psOP")
                            nc.tensor.matmul(
                                psOPs[h][:],
                                UT_sb[h // 2][po : po + 64, t * 128 : t * 128 + 128],
                                wo_sb[po : po + 64, h // 2, et * 512 : et * 512 + 512],
                                start=True, stop=True,
                            )
                        for h in range(HPC):
                            rinv = rinvT_sb[:, t, h : h + 1]
                            if h == 0:
                                nc.vector.tensor_scalar_mul(oslice, psOPs[h][:], rinv)
                            else:
                                nc.vector.scalar_tensor_tensor(
                                    oslice, psOPs[h][:], rinv, oslice,
                                    op0=mybir.AluOpType.mult,
                                    op1=mybir.AluOpType.add,
                                )
                    nc.sync.dma_start(out_p[t * 128 : t * 128 + 128, :], ob[:])

    _split_excess_waits(nc)
    return nc


def _classify_mask(mask):
    m2 = np.asarray(mask).reshape(S, S)
    if np.all(m2 != 0):
        return "dense"
    if np.array_equal(m2 != 0, np.tril(np.ones((S, S), bool))):
        return "causal"
    return "general"


def _make_patterns():
    ii = np.arange(128)[:, None]
    jj = np.arange(512)[None, :]
    p1 = np.zeros((4, 128, 512), np.float32)
    p2 = np.zeros((4, 128, 512), np.float32)
    for k in range(4):
        p1[k] = np.where(jj > ii + 128 * k, -1e30, 0.0)
        # layout-2 tile: partition=j (128), free=i (512)
        p2[k] = np.where(ii + 128 * k > jj, -1e30, 0.0)
    return (
        p1.astype(ml_dtypes.bfloat16),
        p2.astype(ml_dtypes.bfloat16),
    )


def kernel(q, k, v, mask, W_q, b_q, W_k, b_k, W_v, b_v, W_o, b_o):
    q = np.ascontiguousarray(np.asarray(q, np.float32))
    k = np.ascontiguousarray(np.asarray(k, np.float32))
    v = np.ascontiguousarray(np.asarray(v, np.float32))
    W_q = np.asarray(W_q, np.float32)
    W_k = np.asarray(W_k, np.float32)
    W_v = np.asarray(W_v, np.float32)
    W_o = np.asarray(W_o, np.float32)
    b_q = np.asarray(b_q, np.float32)
    b_k = np.asarray(b_k, np.float32)
    b_v = np.asarray(b_v, np.float32)
    b_o = np.asarray(b_o, np.float32)

    mode = _classify_mask(mask)
    nc = _build(mode)

    ident = np.eye(128, dtype=ml_dtypes.bfloat16)
    if mode == "causal":
        patt1, patt2 = _make_patterns()
    elif mode == "general":
        m2 = np.asarray(mask).reshape(S, S)
        mb1 = np.where(m2 != 0, 0.0, -1e30).astype(ml_dtypes.bfloat16)
        mb2 = np.ascontiguousarray(mb1.T)

    dt = _NP_DT[MM_DT]
    in_maps = []
    for c in range(N_CORES):
        bb, hg = c // 4, c % 4
        cols = slice(hg * C_PC, hg * C_PC + C_PC)
        im = {
            "qT": np.ascontiguousarray(q[bb].T).astype(dt),
            "kT": np.ascontiguousarray(k[bb].T).astype(dt),
            "vT": np.ascontiguousarray(v[bb].T).astype(dt),
            "wqT": np.ascontiguousarray(W_q[cols, :].T).astype(dt),
            "wkT": np.ascontiguousarray(W_k[cols, :].T).astype(dt),
            "wvT": np.ascontiguousarray(W_v[cols, :].T).astype(dt),
            "woT": np.ascontiguousarray(W_o[:, cols].T).astype(dt),
            "bq": np.ascontiguousarray(b_q[cols]).reshape(C_PC, 1),
            "bk": np.ascontiguousarray(b_k[cols]).reshape(C_PC, 1),
            "ident": ident,
        }
        if mode == "causal":
            im["patt1"] = patt1
            im["patt2"] = patt2
        elif mode == "general":
            im["mb1"] = mb1
            im["mb2"] = mb2
        in_maps.append(im)

    kw = {}
    if TRACE and _install_ntff_hook():
        kw = dict(trace=True, trace_cores=[0])
    res = run_bass_kernel_spmd(nc, in_maps, core_ids=list(range(N_CORES)), **kw)
    if TRACE:
        _TRACE_RESULT["res"] = res

    attn = np.empty((B, NUM_HEADS, S, S), np.float32)
    out = np.empty((B, S, D_MODEL), np.float32)
    bias_vec = (W_o @ b_v + b_o).astype(np.float32)
    for bb in range(B):
        acc = None
        for hg in range(4):
            r = res.results[bb * 4 + hg]
            attn[bb, hg * HPC : hg * HPC + HPC] = r["attn_out"]
            acc = r["out_p"] if acc is None else acc + r["out_p"]
        out[bb] = acc + bias_vec
    return out, attn
